# revision 8
# baseline (speedup 1.0000x reference)
"""Trainium2 Bass kernel for ComplexLinearAndLeakyReLU.

Math: the reference's basis-conjugated transform collapses to expressions in
a single unit vector t = (U_z, V_z=0, n_z) per (b,n,e):
  t0 = -sign(J2)*sqrt(J0^2+J1^2)/|J|,  t2 = J2/|J|     (computed on HOST)
  p  = t0*X0 + t2*X2
  a  = X - t*p          (A-term)   ->  A@X_i + D@(t_i*p) with D = C - A
  b  = X x t            (B-term)   ->  b0 = X1*t2, b1 = t0*X2 - t2*X0 (split
                                       as B@(t0*X2) + Bn@(t2*X0)), b2 = -X1*t0
                                       (as Bn@(X1*t0)), Bn = -B
  c  = t*p              (C-term)
  Y_i = A@X_i + D@c_i + B/Bn@b_i   (c_1 = 0; a_i + c_i = X_i)
  d = W@Y; out = Y + G*ds, ds = d/32, G = max(-0.8*dot'',0)/(dn''+eps)
  with dot'' = sum_i Y_i*ds_i, dn'' = sum_i ds_i^2  (scale-folded VN relu)

Distribution: data-parallel over batch B=16 -> 2 per core on 8 cores.
Everything on the wire is fp16 (X, t0, t2, weights, out); host casts the
result back to f32. Matmuls fp16 x fp16 -> f32 PSUM.

Runner: program + jitted shard_map executable built once per process and
cached; zero output-donation buffers kept device-resident across calls.
"""

import sys
from concurrent.futures import ThreadPoolExecutor

for _p in ("/opt/trn_rl_repo", "/root/.axon_site/_ro/trn_rl_repo"):
    if _p not in sys.path:
        sys.path.insert(0, _p)

import numpy as np

import concourse.bass as bass
import concourse.tile as tile
from concourse import bacc, mybir

F16 = mybir.dt.float16
F32 = mybir.dt.float32
AF = mybir.ActivationFunctionType

EPS = 1e-6
B, N, E, F = 16, 1024, 256, 256
NCORES = 8
BLOC = B // NCORES          # batches per core
T = 512                     # tokens per super-block
NSB = BLOC * N // T         # super-blocks per core
T3 = 3 * T
DS_SCALE = 1.0 / 32.0       # d is carried as d/32 in fp16

_PROGRAM = None
_RUNTIME = None


def _v3(ap):
    """[128, 3T] AP -> [128, 3, T] view."""
    return ap.rearrange("p (i t) -> p i t", i=3)


def _bcast3(plane_ap):
    """[128, T] AP -> broadcast [128, 3, T] view."""
    return plane_ap.rearrange("p (o t) -> p o t", o=1).broadcast_to([128, 3, T])


def _build_program(repeat=1):
    nc = bacc.Bacc(trn_type="TRN2", target_bir_lowering=False, debug=False)

    Xd = nc.declare_dram_parameter("X", [BLOC, E, 3, N], F16, isOutput=False)
    T0d = nc.declare_dram_parameter("T0", [BLOC, E, N], F16, isOutput=False)
    T2d = nc.declare_dram_parameter("T2", [BLOC, E, N], F16, isOutput=False)
    WAd = nc.declare_dram_parameter("WA", [E, F], F16, isOutput=False)
    WBd = nc.declare_dram_parameter("WB", [E, F], F16, isOutput=False)
    WNd = nc.declare_dram_parameter("WBn", [E, F], F16, isOutput=False)
    WDd = nc.declare_dram_parameter("WD", [E, F], F16, isOutput=False)
    WWd = nc.declare_dram_parameter("WW", [F, F], F16, isOutput=False)
    Od = nc.declare_dram_parameter("out", [BLOC, F, 3, N], F16, isOutput=True)

    vt = nc.vector
    gp = nc.gpsimd
    sc = nc.scalar

    with tile.TileContext(nc) as tc:
        with (
            tc.tile_pool(name="wts", bufs=1) as wpool,
            tc.tile_pool(name="io", bufs=2) as io,
            tc.tile_pool(name="fr", bufs=2) as fr,
            tc.tile_pool(name="yp", bufs=2) as yp,
            tc.tile_pool(name="ep", bufs=2) as ep,
            tc.tile_pool(name="ot", bufs=2) as otp,
            tc.tile_pool(name="psy", bufs=2, space="PSUM") as psy,
            tc.tile_pool(name="psd", bufs=2, space="PSUM") as psd,
        ):
            # ---- replicated weights: lhsT tiles [e_chunk 128, F] ----
            wmats = {}
            for nm, dram in (("A", WAd), ("B", WBd), ("N", WNd),
                             ("D", WDd), ("W", WWd)):
                per_c = []
                for c in range(2):
                    w = wpool.tile([128, F], F16, tag=f"w{nm}{c}")
                    sc.dma_start(w[:], dram[128 * c:128 * (c + 1), :])
                    per_c.append(w)
                wmats[nm] = per_c

            def stage_a(sb):
                """DMA in, front-end, matmuls, psum copies, gp chains."""
                b = sb // (N // T)
                n0 = (sb % (N // T)) * T

                Xt = io.tile([128, 2, 3, T], F16, tag="X")
                Tt = io.tile([128, 2, 2, T], F16, tag="T")
                for c in range(2):
                    e0 = 128 * c
                    nc.sync.dma_start(Xt[:, c], Xd[b, e0:e0 + 128, :, n0:n0 + T])
                    nc.sync.dma_start(Tt[:, c, 0], T0d[b, e0:e0 + 128, n0:n0 + T])
                    nc.sync.dma_start(Tt[:, c, 1], T2d[b, e0:e0 + 128, n0:n0 + T])

                X0 = Xt[:, :, 0, :]
                X1 = Xt[:, :, 1, :]
                X2 = Xt[:, :, 2, :]
                t0 = Tt[:, :, 0, :]
                t2 = Tt[:, :, 1, :]

                # ---- front end: 9 DVE ops on [128, 2, T] fp16 views ----
                pm0 = fr.tile([128, 2, T], F16, tag="pm0")
                pm2 = fr.tile([128, 2, T], F16, tag="pm2")
                p = fr.tile([128, 2, T], F16, tag="p")
                c0 = fr.tile([128, 2, T], F16, tag="c0")
                c2 = fr.tile([128, 2, T], F16, tag="c2")
                b0 = fr.tile([128, 2, T], F16, tag="b0")
                m01 = fr.tile([128, 2, T], F16, tag="m01")
                q2 = fr.tile([128, 2, T], F16, tag="q2")
                q0 = fr.tile([128, 2, T], F16, tag="q0")

                vt.tensor_mul(pm0[:], t0, X0)
                vt.tensor_mul(pm2[:], t2, X2)
                vt.tensor_add(p[:], pm0[:], pm2[:])
                vt.tensor_mul(c0[:], t0, p[:])
                vt.tensor_mul(c2[:], t2, p[:])
                vt.tensor_mul(b0[:], X1, t2)
                vt.tensor_mul(m01[:], X1, t0)
                vt.tensor_mul(q2[:], t0, X2)
                vt.tensor_mul(q0[:], t2, X0)

                # ---- matmul 1: Y[f, (i, tok)] per output chunk m ----
                terms = [
                    [("A", lambda c: Xt[:, c, 0, :]), ("D", lambda c: c0[:, c, :]),
                     ("B", lambda c: b0[:, c, :])],
                    [("A", lambda c: Xt[:, c, 1, :]), ("B", lambda c: q2[:, c, :]),
                     ("N", lambda c: q0[:, c, :])],
                    [("A", lambda c: Xt[:, c, 2, :]), ("D", lambda c: c2[:, c, :]),
                     ("N", lambda c: m01[:, c, :])],
                ]
                ytiles = []
                for m in range(2):
                    ym = yp.tile([128, T3], F16, tag=f"y{m}")
                    for i in range(3):
                        py = psy.tile([128, T], F32, tag="py")
                        k = 0
                        for wtag, rhs in terms[i]:
                            for c in range(2):
                                nc.tensor.matmul(
                                    py[:],
                                    wmats[wtag][c][:, m * 128:(m + 1) * 128],
                                    rhs(c),
                                    start=(k == 0), stop=(k == 5))
                                k += 1
                        sc.activation(ym[:, i * T:(i + 1) * T], py[:], AF.Copy)
                    ytiles.append(ym)

                # ---- matmul 2, psum->sbuf ds copy, gp dot/dn chains ----
                pend = []
                for g in range(2):
                    pd = psd.tile([128, T3], F32, tag="pd")
                    for i in range(3):
                        for c in range(2):
                            nc.tensor.matmul(
                                pd[:, i * T:(i + 1) * T],
                                wmats["W"][c][:, g * 128:(g + 1) * 128],
                                ytiles[c][:, i * T:(i + 1) * T],
                                start=(c == 0), stop=(c == 1))

                    dsb = ep.tile([128, T3], F16, tag=f"dsb{g}")
                    sc.activation(dsb[:], pd[:], AF.Copy, scale=DS_SCALE)

                    def dpl(i):
                        return dsb[:, i * T:(i + 1) * T]

                    def ypl(i):
                        return ytiles[g][:, i * T:(i + 1) * T]

                    # |ds|^2 partials and dot chain on gpsimd (fp16 out)
                    sq0 = ep.tile([128, T], F16, tag=f"sq0{g}")
                    sq1 = ep.tile([128, T], F16, tag=f"sq1{g}")
                    sq2 = ep.tile([128, T], F16, tag=f"sq2{g}")
                    s01 = ep.tile([128, T], F16, tag=f"s01{g}")
                    dnsum = ep.tile([128, T], F16, tag=f"dns{g}")
                    gp.tensor_mul(sq0[:], dpl(0), dpl(0))
                    gp.tensor_mul(sq1[:], dpl(1), dpl(1))
                    gp.tensor_mul(sq2[:], dpl(2), dpl(2))
                    gp.tensor_add(s01[:], sq0[:], sq1[:])
                    gp.tensor_add(dnsum[:], s01[:], sq2[:])

                    pr0 = ep.tile([128, T], F16, tag=f"pr0{g}")
                    pr1 = ep.tile([128, T], F16, tag=f"pr1{g}")
                    pr2 = ep.tile([128, T], F16, tag=f"pr2{g}")
                    s2 = ep.tile([128, T], F16, tag=f"s2{g}")
                    dot = ep.tile([128, T], F16, tag=f"dot{g}")
                    gp.tensor_mul(pr0[:], ypl(0), dpl(0))
                    gp.tensor_mul(pr1[:], ypl(1), dpl(1))
                    gp.tensor_mul(pr2[:], ypl(2), dpl(2))
                    gp.tensor_add(s2[:], pr0[:], pr1[:])
                    gp.tensor_add(dot[:], s2[:], pr2[:])

                    pend.append((dsb, dnsum, dot, ytiles[g]))
                return (b, n0, pend)

            def stage_b(state):
                """DVE epilogue + output DMA (runs one superblock behind)."""
                b, n0, pend = state
                for g in range(2):
                    dsb, dnsum, dot, ym = pend[g]
                    dne = ep.tile([128, T], F32, tag=f"dne{g}")
                    vt.tensor_scalar_add(dne[:], dnsum[:], 1e-9)
                    rcd = ep.tile([128, T], F32, tag=f"rcd{g}")
                    vt.reciprocal_approx_fast(rcd[:], dne[:])
                    gg = ep.tile([128, T], F32, tag=f"gg{g}")
                    vt.tensor_scalar(gg[:], dot[:], -0.8, 0.0,
                                     op0=mybir.AluOpType.mult,
                                     op1=mybir.AluOpType.max)
                    ggh = ep.tile([128, T], F16, tag=f"ggh{g}")
                    vt.tensor_mul(ggh[:], gg[:], rcd[:])

                    # out = Y + G*ds: g0 on DVE, g1 on gpsimd (balance)
                    eng = vt if g == 0 else gp
                    tmp = ep.tile([128, T3], F16, tag=f"tmp{g}")
                    eng.tensor_mul(_v3(tmp[:]), _v3(dsb[:]), _bcast3(ggh[:]))
                    ot = otp.tile([128, T3], F16, tag=f"o{g}")
                    eng.tensor_add(_v3(ot[:]), _v3(tmp[:]), _v3(ym[:]))
                    nc.sync.dma_start(
                        Od[b, g * 128:(g + 1) * 128, :, n0:n0 + T], ot[:])

            # ---- software-pipelined driver: stage B runs one sb behind ----
            pending = None
            for sb in range(NSB * repeat + 1):
                nxt = stage_a(sb % NSB) if sb < NSB * repeat else None
                if pending is not None:
                    stage_b(pending)
                pending = nxt

    nc.finalize()
    return nc


def _get_program():
    global _PROGRAM
    if _PROGRAM is None:
        _PROGRAM = _build_program()
    return _PROGRAM


# ---------------------------------------------------------------------------
# host-side preprocessing
# ---------------------------------------------------------------------------

def _prep_slice(X, J, X16, T0, T2, b):
    """Fill X16[b], T0[b], T2[b] from X[b], J[b] (f32 math, fp16 out)."""
    np.copyto(X16[b], X[b].transpose(1, 2, 0))     # [N,E,3] -> [E,3,N]
    jj = J[b]                                       # [N, E, 3]
    sq = jj * jj
    q01 = sq[..., 0] + sq[..., 1]
    jsq = q01 + sq[..., 2]
    nrm = np.sqrt(jsq)
    rn = 1.0 / (nrm + EPS)
    j2 = jj[..., 2]
    sgn = np.where(j2 + EPS * (nrm + EPS) >= 0, np.float32(1.0),
                   np.float32(-1.0))
    t0 = -sgn * np.sqrt(q01) * rn                   # [N, E]
    t2 = j2 * rn
    np.copyto(T0[b], t0.T)
    np.copyto(T2[b], t2.T)


def prepare_global_inputs(X, J, A, Bw, Cw, W):
    """Full inputs -> dict of global (concat-over-cores) device arrays."""
    X = np.asarray(X, np.float32)
    J = np.asarray(J, np.float32)
    A = np.asarray(A, np.float32)
    Bw = np.asarray(Bw, np.float32)
    Cw = np.asarray(Cw, np.float32)
    W = np.asarray(W, np.float32)

    X16 = np.empty((B, E, 3, N), np.float16)
    T0 = np.empty((B, E, N), np.float16)
    T2 = np.empty((B, E, N), np.float16)
    with ThreadPoolExecutor(max_workers=16) as ex:
        list(ex.map(lambda b: _prep_slice(X, J, X16, T0, T2, b), range(B)))

    def rep(w):  # replicate per-core weight along concat axis
        w16 = np.ascontiguousarray(w, np.float16)
        return np.broadcast_to(w16, (NCORES,) + w16.shape).reshape(
            NCORES * w16.shape[0], w16.shape[1])

    return {
        "X": X16, "T0": T0, "T2": T2,
        "WA": rep(A.T), "WB": rep(Bw.T), "WBn": rep(-Bw.T),
        "WD": rep((Cw - A).T), "WW": rep(W.T),
    }


def prepare_shard_inputs(X, J, A, Bw, Cw, W):
    """Per-core in_maps (for harnesses that want the spmd-style list)."""
    g = prepare_global_inputs(X, J, A, Bw, Cw, W)
    maps = []
    for d in range(NCORES):
        m = {}
        for k, v in g.items():
            n0 = v.shape[0] // NCORES
            m[k] = v[d * n0:(d + 1) * n0]
        maps.append(m)
    return maps


# ---------------------------------------------------------------------------
# persistent runner
# ---------------------------------------------------------------------------

def _get_runtime():
    global _RUNTIME
    if _RUNTIME is not None:
        return _RUNTIME

    import jax
    from jax.sharding import Mesh, PartitionSpec, NamedSharding
    from jax.experimental.shard_map import shard_map
    import concourse.bass2jax as b2j

    nc = _get_program()
    b2j.install_neuronx_cc_hook()

    pname = nc.partition_id_tensor.name if nc.partition_id_tensor else None
    in_names, out_names, out_avals, zeros = [], [], [], []
    for alloc in nc.m.functions[0].allocations:
        if not isinstance(alloc, mybir.MemoryLocationSet):
            continue
        name = alloc.memorylocations[0].name
        if alloc.kind == "ExternalInput":
            if name != pname:
                in_names.append(name)
        elif alloc.kind == "ExternalOutput":
            out_names.append(name)
            shape, dtype = tuple(alloc.tensor_shape), mybir.dt.np(alloc.dtype)
            out_avals.append(jax.core.ShapedArray(shape, dtype))
            zeros.append(np.zeros(shape, dtype))
    all_in = in_names + out_names + ([pname] if pname else [])
    n_par, n_out = len(in_names), len(out_avals)

    def _body(*args):
        ops = list(args)
        if pname:
            ops.append(b2j.partition_id_tensor())
        return tuple(b2j._bass_exec_p.bind(
            *ops, out_avals=tuple(out_avals), in_names=tuple(all_in),
            out_names=tuple(out_names), lowering_input_output_aliases=(),
            sim_require_finite=True, sim_require_nnan=True, nc=nc))

    mesh = Mesh(np.asarray(jax.devices()[:NCORES]), ("core",))
    fn = jax.jit(shard_map(_body, mesh=mesh,
                           in_specs=(PartitionSpec("core"),) * (n_par + n_out),
                           out_specs=(PartitionSpec("core"),) * n_out,
                           check_rep=False), keep_unused=True)
    sharding = NamedSharding(mesh, PartitionSpec("core"))
    dzeros = [jax.device_put(
        np.zeros((NCORES * z.shape[0],) + z.shape[1:], z.dtype), sharding)
        for z in zeros]
    for z in dzeros:
        jax.block_until_ready(z)
    _RUNTIME = (jax, fn, sharding, in_names, out_names, dzeros)
    return _RUNTIME


def kernel(X, J, A, Bw, Cw, W, device=None, **_unused):
    jax, fn, sharding, in_names, out_names, dzeros = _get_runtime()
    g = prepare_global_inputs(X, J, A, Bw, Cw, W)
    din = [jax.device_put(g[nm], sharding) for nm in in_names]
    outs = fn(*din, *dzeros)
    out16 = np.asarray(outs[out_names.index("out")])   # [B, F, 3, N] fp16
    return out16.astype(np.float32)
